# revision 13
# baseline (speedup 1.0000x reference)
"""Bass/Tile Trainium2 kernel for dense causal multi-head attention.

Problem: x[2,2048,1024] -> qkv (w_qkv [3072,1024]) -> 16-head causal
attention -> out proj (w_proj [1024,1024], b_proj) -> [2,2048,1024].

Sharding over 8 NeuronCores: data-parallel over batch (2) x
tensor-parallel over heads (4 groups of 4 heads). Each core computes its
768-row slice of the qkv projection, causal attention for its 4 heads,
and a partial output projection over its 256 head-dim columns. The
all-reduce after proj is realized host-side at gather time (sum of 4
partials per batch) together with the bias add.

On-core layout: activations kept transposed ([feature, seq]) so that
  * scores are computed directly as S^T = K_tile^T-stationary @ Q-moving
    (no P transposes anywhere),
  * softmax reduction over keys happens via a ones-column appended to V
    (denominator falls out of the same PE accumulation as attn@V),
  * head pairs sit at partition offsets 0/64 and their K=64 score
    matmuls run concurrently in different PE row groups.
Matmuls run in float32r (fp32 storage, ~1e-4 matmul precision, 4x the
fp32 PE rate); everything else is plain fp32.
"""

import sys
from contextlib import ExitStack

if "/opt/trn_rl_repo" not in sys.path:
    sys.path.insert(0, "/opt/trn_rl_repo")

import numpy as np

import concourse.bass as bass
import concourse.tile as tile
from concourse import bacc, mybir
from concourse.bass_utils import run_bass_kernel_spmd
from concourse.masks import make_identity

F32 = mybir.dt.float32
F32R = mybir.dt.float32r
AF = mybir.ActivationFunctionType

B, N, C = 2, 2048, 1024
H_TOT, D = 16, 64
NCORES = 8
HPC = H_TOT // (NCORES // B)  # heads per core = 4
HD = HPC * D                  # 256 per-core head-dim columns
CT = C // 128                 # 8 contraction tiles
NT = N // 128                 # 16 seq tiles
QCH = N // 512                # 4 query chunks of 512
SCALE = float(D) ** -0.5


def _r(ap):
    return ap.bitcast(F32R)


def build_nc():
    nc = bacc.Bacc("TRN2", target_bir_lowering=False, debug=False)
    xT = nc.dram_tensor("xT", [C, N], F32R, kind="ExternalInput").ap()
    wqkvT = nc.dram_tensor("wqkvT", [C, 3 * HD], F32R, kind="ExternalInput").ap()
    wpT = nc.dram_tensor("wpT", [HD, C], F32R, kind="ExternalInput").ap()
    out = nc.dram_tensor("out", [N, C], F32, kind="ExternalOutput").ap()

    xT_r = xT.rearrange("(ct p) n -> ct p n", p=128)
    wq_r = wqkvT.rearrange("(ct p) j -> ct p j", p=128)
    wp_r = wpT.rearrange("(ht p) co -> ht p co", p=128)
    out_r = out.rearrange("(nt p) co -> nt p co", p=128)

    with tile.TileContext(nc) as tc, ExitStack() as ctx:
        const = ctx.enter_context(tc.tile_pool(name="const", bufs=1))
        qkv_pool = ctx.enter_context(tc.tile_pool(name="qkv", bufs=1))
        yT_pool = ctx.enter_context(tc.tile_pool(name="yT", bufs=1))

        identity = const.tile([128, 128], F32, tag="id")
        make_identity(nc, identity[:])
        ones64 = const.tile([1, 64], F32, tag="ones64")
        nc.vector.memset(ones64[:], 1.0)

        # qkv_T tiles [128, N]: jt 0/1 = q heads (01)/(23); 2/3 = k; 4/5 = v.
        qkvT = [qkv_pool.tile([128, N], F32R if jt < 4 else F32,
                      tag=f"qkv{jt}", name=f"qkvT{jt}")
                for jt in range(6)]
        yT = yT_pool.tile([128, 2, N], F32R, tag="yT")

        # ---- Phase 1: qkv_T = wqkvT.T-stationary @ xT-moving ----
        with tc.tile_pool(name="x", bufs=1) as x_pool, \
             tc.tile_pool(name="wq", bufs=1) as w_pool, \
             tc.tile_pool(name="mmps", bufs=3, space="PSUM") as mm_ps:
            xt = [x_pool.tile([128, N], F32R, tag=f"x{ct}", name=f"xt{ct}") for ct in range(CT)]
            wq = [w_pool.tile([128, 3 * HD], F32R, tag=f"w{ct}", name=f"wq{ct}") for ct in range(CT)]
            for ct in range(CT):
                nc.sync.dma_start(wq[ct][:], wq_r[ct])
                nc.sync.dma_start(xt[ct][:], xT_r[ct])
            for jt in range(6):
                for nch in range(QCH):
                    ps = mm_ps.tile([128, 512], F32, tag="mm")
                    for ct in range(CT):
                        nc.tensor.matmul(
                            ps[:],
                            wq[ct][:, jt * 128:(jt + 1) * 128],
                            xt[ct][:, nch * 512:(nch + 1) * 512],
                            start=(ct == 0), stop=(ct == CT - 1),
                        )
                    nc.scalar.activation(
                        qkvT[jt][:, nch * 512:(nch + 1) * 512], ps[:], AF.Copy)

        with tc.tile_pool(name="v", bufs=1) as v_pool, \
             tc.tile_pool(name="mask", bufs=1) as mask_pool, \
             tc.tile_pool(name="p", bufs=4) as p_pool, \
             tc.tile_pool(name="recip", bufs=3) as r_pool, \
             tc.tile_pool(name="bcast", bufs=3) as bc_pool:
            # V per head: [k-partition, kt, 65]; col 64 = ones (denominator).
            v_sb = [v_pool.tile([128, NT, 65], F32R, tag=f"v{h}", name=f"v{h}")
                    for h in range(HPC)]
            # Causal masks for the 4 diagonal positions of a 512-query chunk.
            masks = mask_pool.tile([128, 4, 512], F32, tag="mask")
            for r in range(4):
                m = masks[:, r, :]
                nc.gpsimd.memset(m, 1.0)
                # keep where q_local - k_local >= 0: y - x - 128*r >= 0
                nc.gpsimd.affine_select(
                    out=m, in_=m, compare_op=mybir.AluOpType.is_ge, fill=0.0,
                    base=-128 * r, channel_multiplier=-1, pattern=[[1, 512]],
                )
            onescol = mask_pool.tile([128, NT], F32, tag="onescol")
            nc.vector.memset(onescol[:], 1.0)
            for h in range(HPC):
                nc.vector.tensor_copy(v_sb[h][:, :, 64], onescol[:])

            # ---- Phase 1.5: V = transpose(v_T) per head ----
            with tc.tile_pool(name="trps", bufs=3, space="PSUM") as tr_ps:
                for hp in range(2):
                    for nt in range(NT):
                        pst = tr_ps.tile([128, 128], F32, tag="tr")
                        nc.tensor.transpose(
                            pst[:], qkvT[4 + hp][:, nt * 128:(nt + 1) * 128],
                            identity[:])
                        nc.vector.tensor_copy(
                            v_sb[2 * hp][:, nt, 0:64], pst[:, 0:64])
                        nc.vector.tensor_copy(
                            v_sb[2 * hp + 1][:, nt, 0:64], pst[:, 64:128])

            # ---- Phase 2: causal attention, head-pair at a time ----
            with tc.tile_pool(name="sps", bufs=3, space="PSUM") as s_ps, \
                 tc.tile_pool(name="avps", bufs=2, space="PSUM") as av_ps, \
                 tc.tile_pool(name="bcps", bufs=1, space="PSUM") as bc_ps:
                attention_phase(nc, tc, qkvT, v_sb, masks, yT, ones64,
                                s_ps, av_ps, bc_ps, p_pool, r_pool, bc_pool)

        # ---- Phase 3: partial out-projection ----
        with tc.tile_pool(name="prps", bufs=4, space="PSUM") as pr_ps, \
             tc.tile_pool(name="wp", bufs=1) as wp_pool, \
             tc.tile_pool(name="o", bufs=4) as o_pool:
            wp = [wp_pool.tile([128, C], F32R, tag=f"wp{ht}", name=f"wp{ht}")
                  for ht in range(2)]
            for ht in range(2):
                nc.sync.dma_start(wp[ht][:], wp_r[ht])
            for nt in range(NT):
                pso = [pr_ps.tile([128, 512], F32, tag="pr", name="pso")
                       for _ in range(2)]
                for ht in range(2):
                    for cok in range(2):
                        nc.tensor.matmul(
                            pso[cok][:],
                            yT[:, ht, nt * 128:(nt + 1) * 128],
                            wp[ht][:, cok * 512:(cok + 1) * 512],
                            start=(ht == 0), stop=(ht == 1),
                        )
                for cok in range(2):
                    ot = o_pool.tile([128, 512], F32, tag="ot")
                    nc.scalar.activation(ot[:], pso[cok][:], AF.Copy)
                    nc.sync.dma_start(
                        out_r[nt, :, cok * 512:(cok + 1) * 512], ot[:])

    nc.compile()
    return nc


def attention_phase(nc, tc, qkvT, v_sb, masks, yT, ones64, s_ps, av_ps,
                    bc_ps, p_pool, r_pool, bc_pool):
    for hp in range(2):
                q_t, k_t = qkvT[hp], qkvT[2 + hp]
                for qc in range(QCH):
                    nkt = 4 * (qc + 1)
                    qs = slice(qc * 512, (qc + 1) * 512)
                    pav = [av_ps.tile([65, 512], F32, tag=f"av{po}", name=f"pav{po}")
                           for po in range(2)]
                    for kt in range(nkt):
                        pts = []
                        for po in range(2):
                            o = 64 * po
                            pss = s_ps.tile([128, 512], F32, tag="s")
                            nc.tensor.matmul(
                                pss[:],
                                k_t[o:o + 64, kt * 128:(kt + 1) * 128],
                                q_t[o:o + 64, qs],
                                start=True, stop=True,
                            )
                            pt = p_pool.tile([128, 512], F32R, tag="pt")
                            nc.scalar.activation(pt[:], pss[:], AF.Exp, scale=SCALE)
                            if kt >= 4 * qc:
                                nc.vector.tensor_mul(
                                    pt[:], pt[:], masks[:, kt - 4 * qc, :])
                            pts.append(pt)
                        for po in range(2):
                            nc.tensor.matmul(
                                pav[po][:],
                                v_sb[2 * hp + po][:, kt, :],
                                pts[po][:],
                                start=(kt == 0), stop=(kt == nkt - 1),
                            )
                    for po in range(2):
                        rc = r_pool.tile([1, 512], F32, tag="rc")
                        nc.vector.reciprocal(rc[:], pav[po][64:65, :])
                        # broadcast 1/denom across 64 partitions via PE
                        pbc = bc_ps.tile([64, 512], F32, tag="pbc")
                        nc.tensor.matmul(pbc[:], ones64[:], rc[:],
                                         start=True, stop=True)
                        bc = bc_pool.tile([64, 512], F32, tag="bc")
                        nc.vector.tensor_copy(bc[:], pbc[:])
                        nc.vector.tensor_mul(
                            yT[64 * po:64 * po + 64, hp, qs],
                            pav[po][0:64, :], bc[:])


_NC = None


def _get_nc():
    global _NC
    if _NC is None:
        _NC = build_nc()
    return _NC


def make_in_maps(x, w_qkv, w_proj):
    x = np.asarray(x, dtype=np.float32)
    w_qkv = np.asarray(w_qkv, dtype=np.float32)
    w_proj = np.asarray(w_proj, dtype=np.float32)
    xT = [np.ascontiguousarray(x[b].T) for b in range(B)]
    in_maps = []
    for c in range(NCORES):
        b, g = divmod(c, NCORES // B)
        rows = np.concatenate(
            [np.arange(s * C + g * HD, s * C + (g + 1) * HD) for s in range(3)])
        wqkvT = np.ascontiguousarray(w_qkv[rows, :].T)
        wpT = np.ascontiguousarray(w_proj[:, g * HD:(g + 1) * HD].T)
        in_maps.append({"xT": xT[b], "wqkvT": wqkvT, "wpT": wpT})
    return in_maps


def assemble(results, b_proj):
    b_proj = np.asarray(b_proj, dtype=np.float32)
    out = np.zeros((B, N, C), dtype=np.float32)
    for c in range(NCORES):
        b = c // (NCORES // B)
        out[b] += results[c]["out"]
    out += b_proj[None, None, :]
    return out


def kernel(x, w_qkv, w_proj, b_proj):
    nc = _get_nc()
    in_maps = make_in_maps(x, w_qkv, w_proj)
    res = run_bass_kernel_spmd(nc, in_maps, core_ids=list(range(NCORES)))
    return assemble(res.results, b_proj)


# revision 18
# speedup vs baseline: 1.2599x; 1.2599x over previous
"""Bass/Tile Trainium2 kernel for dense causal multi-head attention.

Problem: x[2,2048,1024] -> qkv (w_qkv [3072,1024]) -> 16-head causal
attention -> out proj (w_proj [1024,1024], b_proj) -> [2,2048,1024].

Sharding over 8 NeuronCores: data-parallel over batch (2) x
tensor-parallel over heads (4 groups of 4 heads). Each core computes its
768-row slice of the qkv projection, causal attention for its 4 heads,
and a partial output projection over its 256 head-dim columns. The
all-reduce after proj is realized host-side at gather time (sum of 4
partials per batch) together with the bias add.

On-core layout: activations kept transposed ([feature, seq]) so that
  * scores are computed directly as S^T = K_tile^T-stationary @ Q-moving
    (no P transposes anywhere),
  * softmax reduction over keys happens via a ones-column appended to V
    (denominator falls out of the same PE accumulation as attn@V),
  * head pairs sit at partition offsets 0/64 and their K=64 score
    matmuls run concurrently in different PE row groups.
Matmuls run in float32r (fp32 storage, ~1e-4 matmul precision, 4x the
fp32 PE rate); everything else is plain fp32.
"""

import sys
from contextlib import ExitStack

if "/opt/trn_rl_repo" not in sys.path:
    sys.path.insert(0, "/opt/trn_rl_repo")

import numpy as np

import concourse.bass as bass
import concourse.tile as tile
from concourse import bacc, mybir
from concourse.bass_utils import run_bass_kernel_spmd
from concourse.masks import make_identity

F32 = mybir.dt.float32
F32R = mybir.dt.float32r
AF = mybir.ActivationFunctionType

B, N, C = 2, 2048, 1024
H_TOT, D = 16, 64
NCORES = 8
HPC = H_TOT // (NCORES // B)  # heads per core = 4
HD = HPC * D                  # 256 per-core head-dim columns
CT = C // 128                 # 8 contraction tiles
NT = N // 128                 # 16 seq tiles
QCH = N // 512                # 4 query chunks of 512
SCALE = float(D) ** -0.5


def _r(ap):
    return ap.bitcast(F32R)


def build_nc():
    nc = bacc.Bacc("TRN2", target_bir_lowering=False, debug=False)
    xT = nc.dram_tensor("xT", [C, N], F32R, kind="ExternalInput").ap()
    wqkvT = nc.dram_tensor("wqkvT", [C, 3 * HD], F32R, kind="ExternalInput").ap()
    wpT = nc.dram_tensor("wpT", [HD, C], F32R, kind="ExternalInput").ap()
    out = nc.dram_tensor("out", [N, C], F32, kind="ExternalOutput").ap()

    xT_r = xT.rearrange("(ct p) n -> ct p n", p=128)
    wq_r = wqkvT.rearrange("(ct p) j -> ct p j", p=128)
    wp_r = wpT.rearrange("(ht p) co -> ht p co", p=128)
    out_r = out.rearrange("(nt p) co -> nt p co", p=128)

    with tile.TileContext(nc) as tc, ExitStack() as ctx:
        const = ctx.enter_context(tc.tile_pool(name="const", bufs=1))
        qkv_pool = ctx.enter_context(tc.tile_pool(name="qkv", bufs=1))
        yT_pool = ctx.enter_context(tc.tile_pool(name="yT", bufs=1))

        identity = const.tile([128, 128], F32, tag="id")
        make_identity(nc, identity[:])
        ones64 = const.tile([128, 64], F32, tag="ones64")
        nc.vector.memset(ones64[:], 1.0)

        # qkv_T tiles [128, N]: jt 0/1 = q heads (01)/(23); 2/3 = k; 4/5 = v.
        qkvT = [qkv_pool.tile([128, N], F32R if jt < 4 else F32,
                      tag=f"qkv{jt}", name=f"qkvT{jt}")
                for jt in range(6)]
        yT = yT_pool.tile([128, 2, N], F32R, tag="yT")

        # ---- Phase 1: qkv_T = wqkvT.T-stationary @ xT-moving ----
        with tc.tile_pool(name="x", bufs=1) as x_pool, \
             tc.tile_pool(name="wq", bufs=1) as w_pool, \
             tc.tile_pool(name="mmps", bufs=3, space="PSUM") as mm_ps:
            xt = [x_pool.tile([128, N], F32R, tag=f"x{ct}", name=f"xt{ct}") for ct in range(CT)]
            wq = [w_pool.tile([128, 3 * HD], F32R, tag=f"w{ct}", name=f"wq{ct}") for ct in range(CT)]
            for ct in range(CT):
                nc.sync.dma_start(wq[ct][:], wq_r[ct])
                nc.sync.dma_start(xt[ct][:], xT_r[ct])
            for jt in range(6):
                for nch in range(QCH):
                    ps = mm_ps.tile([128, 512], F32, tag="mm")
                    for ct in range(CT):
                        nc.tensor.matmul(
                            ps[:],
                            wq[ct][:, jt * 128:(jt + 1) * 128],
                            xt[ct][:, nch * 512:(nch + 1) * 512],
                            start=(ct == 0), stop=(ct == CT - 1),
                        )
                    nc.scalar.activation(
                        qkvT[jt][:, nch * 512:(nch + 1) * 512], ps[:], AF.Copy)

        with tc.tile_pool(name="v", bufs=1) as v_pool, \
             tc.tile_pool(name="mask", bufs=1) as mask_pool, \
             tc.tile_pool(name="p", bufs=6) as p_pool, \
             tc.tile_pool(name="avsb", bufs=3) as r_pool, \
             tc.tile_pool(name="bcast", bufs=3) as bc_pool:
            # V per head: [k-partition, kt, 65]; col 64 = ones (denominator).
            v_sb = [v_pool.tile([128, NT, 65], F32R, tag=f"v{h}", name=f"v{h}")
                    for h in range(HPC)]
            # Causal masks for the 4 diagonal positions of a 512-query chunk.
            masks = mask_pool.tile([128, 4, 512], F32, tag="mask")
            for r in range(4):
                m = masks[:, r, :]
                nc.gpsimd.memset(m, 1.0)
                # keep where q_local - k_local >= 0: y - x - 128*r >= 0
                nc.gpsimd.affine_select(
                    out=m, in_=m, compare_op=mybir.AluOpType.is_ge, fill=0.0,
                    base=-128 * r, channel_multiplier=-1, pattern=[[1, 512]],
                )
            onescol = mask_pool.tile([128, NT], F32, tag="onescol")
            nc.vector.memset(onescol[:], 1.0)
            for h in range(HPC):
                nc.vector.tensor_copy(v_sb[h][:, :, 64], onescol[:])

            # ---- Phase 1.5: V = transpose(v_T) per head ----
            with tc.tile_pool(name="trps", bufs=3, space="PSUM") as tr_ps:
                for hp in range(2):
                    for nt in range(NT):
                        pst = tr_ps.tile([128, 128], F32, tag="tr")
                        nc.tensor.transpose(
                            pst[:], qkvT[4 + hp][:, nt * 128:(nt + 1) * 128],
                            identity[:])
                        nc.vector.tensor_copy(
                            v_sb[2 * hp][:, nt, 0:64], pst[:, 0:64])
                        nc.vector.tensor_copy(
                            v_sb[2 * hp + 1][:, nt, 0:64], pst[:, 64:128])

            # ---- Phase 2: causal attention, head-pair at a time ----
            with tc.tile_pool(name="sps", bufs=3, space="PSUM") as s_ps, \
                 tc.tile_pool(name="avps", bufs=2, space="PSUM") as av_ps, \
                 tc.tile_pool(name="bcps", bufs=1, space="PSUM") as bc_ps:
                attention_phase(nc, tc, qkvT, v_sb, masks, yT, ones64,
                                s_ps, av_ps, bc_ps, p_pool, r_pool, bc_pool)

        # ---- Phase 3: partial out-projection ----
        with tc.tile_pool(name="prps", bufs=4, space="PSUM") as pr_ps, \
             tc.tile_pool(name="wp", bufs=1) as wp_pool, \
             tc.tile_pool(name="o", bufs=4) as o_pool:
            wp = [wp_pool.tile([128, C], F32R, tag=f"wp{ht}", name=f"wp{ht}")
                  for ht in range(2)]
            for ht in range(2):
                nc.sync.dma_start(wp[ht][:], wp_r[ht])
            for nt in range(NT):
                pso = [pr_ps.tile([128, 512], F32, tag="pr", name="pso")
                       for _ in range(2)]
                for ht in range(2):
                    for cok in range(2):
                        nc.tensor.matmul(
                            pso[cok][:],
                            yT[:, ht, nt * 128:(nt + 1) * 128],
                            wp[ht][:, cok * 512:(cok + 1) * 512],
                            start=(ht == 0), stop=(ht == 1),
                        )
                for cok in range(2):
                    ot = o_pool.tile([128, 512], F32, tag="ot")
                    nc.vector.tensor_copy(ot[:], pso[cok][:])
                    nc.sync.dma_start(
                        out_r[nt, :, cok * 512:(cok + 1) * 512], ot[:])

    nc.compile()
    return nc


def attention_phase(nc, tc, qkvT, v_sb, masks, yT, ones64, s_ps, av_ps,
                    bc_ps, p_pool, r_pool, bc_pool):
    def make_pts(hp, qc, kt, qs):
        """S^T matmuls + exp for one (pair, k-tile); head pair concurrent
        in PE row groups 0-63 / 64-127."""
        q_t, k_t = qkvT[hp], qkvT[2 + hp]
        pss = []
        for po in range(2):
            o = 64 * po
            ps = s_ps.tile([128, 512], F32, tag="s")
            nc.tensor.matmul(
                ps[:],
                k_t[o:o + 64, kt * 128:(kt + 1) * 128],
                q_t[o:o + 64, qs],
                start=True, stop=True,
            )
            pss.append(ps)
        pts = []
        for po in range(2):
            pt = p_pool.tile([128, 512], F32R, tag="pt")
            nc.scalar.activation(pt[:], pss[po][:], AF.Exp, scale=SCALE)
            if kt >= 4 * qc:
                nc.vector.tensor_mul(pt[:], pt[:], masks[:, kt - 4 * qc, :])
            pts.append(pt)
        return pts

    for hp in range(2):
        for qc in range(QCH):
            nkt = 4 * (qc + 1)
            qs = slice(qc * 512, (qc + 1) * 512)
            pav = [av_ps.tile([65, 512], F32, tag=f"av{po}", name=f"pav{po}")
                   for po in range(2)]
            # software pipeline: AV for k-tile kt issues after S for kt+1,
            # so PE never idles waiting on the exp (ScalarE) of the same kt.
            prev = make_pts(hp, qc, 0, qs)
            for kt in range(1, nkt + 1):
                cur = make_pts(hp, qc, kt, qs) if kt < nkt else None
                for po in range(2):
                    nc.tensor.matmul(
                        pav[po][:],
                        v_sb[2 * hp + po][:, kt - 1, :],
                        prev[po][:],
                        start=(kt == 1), stop=(kt == nkt),
                    )
                prev = cur
            for po in range(2):
                # evict accumulator to SBUF immediately (frees the PSUM
                # slot so the next chunk's AV can start), then normalize:
                # rows 0-63 x broadcast(1/row64).
                av = r_pool.tile([65, 512], F32, tag="avsb")
                nc.vector.tensor_copy(av[:], pav[po][:])
                pbc = bc_ps.tile([64, 512], F32, tag="pbc")
                nc.tensor.matmul(pbc[:], ones64[64:65, :], av[64:65, :],
                                 start=True, stop=True)
                bc = bc_pool.tile([64, 512], F32, tag="bc")
                nc.vector.reciprocal_approx_accurate(
                    bc[:], pbc[:], scratch=bc_pool.tile(
                        [64, 512], F32, tag="bcs", name="bcs")[:])
                nc.vector.tensor_mul(
                    yT[64 * po:64 * po + 64, hp, qs],
                    av[0:64, :], bc[:])


_NC = None


def _get_nc():
    global _NC
    if _NC is None:
        _NC = build_nc()
    return _NC


def make_in_maps(x, w_qkv, w_proj):
    x = np.asarray(x, dtype=np.float32)
    w_qkv = np.asarray(w_qkv, dtype=np.float32)
    w_proj = np.asarray(w_proj, dtype=np.float32)
    xT = [np.ascontiguousarray(x[b].T) for b in range(B)]
    in_maps = []
    for c in range(NCORES):
        b, g = divmod(c, NCORES // B)
        rows = np.concatenate(
            [np.arange(s * C + g * HD, s * C + (g + 1) * HD) for s in range(3)])
        wqkvT = np.ascontiguousarray(w_qkv[rows, :].T)
        wpT = np.ascontiguousarray(w_proj[:, g * HD:(g + 1) * HD].T)
        in_maps.append({"xT": xT[b], "wqkvT": wqkvT, "wpT": wpT})
    return in_maps


def assemble(results, b_proj):
    b_proj = np.asarray(b_proj, dtype=np.float32)
    out = np.zeros((B, N, C), dtype=np.float32)
    for c in range(NCORES):
        b = c // (NCORES // B)
        out[b] += results[c]["out"]
    out += b_proj[None, None, :]
    return out


def kernel(x, w_qkv, w_proj, b_proj):
    nc = _get_nc()
    in_maps = make_in_maps(x, w_qkv, w_proj)
    res = run_bass_kernel_spmd(nc, in_maps, core_ids=list(range(NCORES)))
    return assemble(res.results, b_proj)


# revision 20
# speedup vs baseline: 1.3431x; 1.0661x over previous
"""Bass/Tile Trainium2 kernel for dense causal multi-head attention.

Problem: x[2,2048,1024] -> qkv (w_qkv [3072,1024]) -> 16-head causal
attention -> out proj (w_proj [1024,1024], b_proj) -> [2,2048,1024].

Sharding over 8 NeuronCores: data-parallel over batch (2) x
tensor-parallel over heads (4 groups of 4 heads). Each core computes its
768-row slice of the qkv projection, causal attention for its 4 heads,
and a partial output projection over its 256 head-dim columns. The
all-reduce after proj is realized host-side at gather time (sum of 4
partials per batch) together with the bias add.

On-core layout: activations kept transposed ([feature, seq]) so that
  * scores are computed directly as S^T = K_tile^T-stationary @ Q-moving
    (no P transposes anywhere),
  * softmax reduction over keys happens via a ones-column appended to V
    (denominator falls out of the same PE accumulation as attn@V),
  * head pairs sit at partition offsets 0/64 and their K=64 score
    matmuls run concurrently in different PE row groups.
Matmuls run in float32r (fp32 storage, ~1e-4 matmul precision, 4x the
fp32 PE rate); everything else is plain fp32.
"""

import sys
from contextlib import ExitStack

if "/opt/trn_rl_repo" not in sys.path:
    sys.path.insert(0, "/opt/trn_rl_repo")

import numpy as np

import concourse.bass as bass
import concourse.tile as tile
from concourse import bacc, mybir
from concourse.bass_utils import run_bass_kernel_spmd
from concourse.masks import make_identity

F32 = mybir.dt.float32
F32R = mybir.dt.float32r
AF = mybir.ActivationFunctionType

B, N, C = 2, 2048, 1024
H_TOT, D = 16, 64
NCORES = 8
HPC = H_TOT // (NCORES // B)  # heads per core = 4
HD = HPC * D                  # 256 per-core head-dim columns
CT = C // 128                 # 8 contraction tiles
NT = N // 128                 # 16 seq tiles
QCH = N // 512                # 4 query chunks of 512
SCALE = float(D) ** -0.5


def _r(ap):
    return ap.bitcast(F32R)


def build_nc():
    nc = bacc.Bacc("TRN2", target_bir_lowering=False, debug=False)
    xT = nc.dram_tensor("xT", [C, N], F32R, kind="ExternalInput").ap()
    wqkvT = nc.dram_tensor("wqkvT", [C, 3 * HD], F32R, kind="ExternalInput").ap()
    wpT = nc.dram_tensor("wpT", [HD, C], F32R, kind="ExternalInput").ap()
    out = nc.dram_tensor("out", [N, C], F32, kind="ExternalOutput").ap()

    xT_r = xT.rearrange("(ct p) n -> ct p n", p=128)
    wq_r = wqkvT.rearrange("(ct p) j -> ct p j", p=128)
    wp_r = wpT.rearrange("(ht p) co -> ht p co", p=128)
    out_r = out.rearrange("(nt p) co -> nt p co", p=128)

    with tile.TileContext(nc) as tc, ExitStack() as ctx:
        const = ctx.enter_context(tc.tile_pool(name="const", bufs=1))
        qkv_pool = ctx.enter_context(tc.tile_pool(name="qkv", bufs=1))
        yT_pool = ctx.enter_context(tc.tile_pool(name="yT", bufs=1))

        identity = const.tile([128, 128], F32, tag="id")
        make_identity(nc, identity[:])
        ones64f = const.tile([128, 64], F32, tag="ones64f")
        nc.vector.memset(ones64f[:], 1.0)
        ones64 = const.tile([128, 64], F32R, tag="ones64")
        nc.vector.tensor_copy(ones64[:], ones64f[:])

        # qkv_T tiles [128, N]: jt 0/1 = q heads (01)/(23); 2/3 = k; 4/5 = v.
        qkvT = [qkv_pool.tile([128, N], F32R if jt < 4 else F32,
                      tag=f"qkv{jt}", name=f"qkvT{jt}")
                for jt in range(6)]
        yT = yT_pool.tile([128, 2, N], F32R, tag="yT")

        v_pool = ctx.enter_context(tc.tile_pool(name="v", bufs=1))
        mask_pool = ctx.enter_context(tc.tile_pool(name="mask", bufs=1))
        # V per head: [k-partition, kt, 65]; col 64 = ones (denominator).
        v_sb = [v_pool.tile([128, NT, 65], F32R, tag=f"v{h}", name=f"v{h}")
                for h in range(HPC)]
        # Causal masks for the 4 diagonal positions of a 512-query chunk.
        masks = mask_pool.tile([128, 4, 512], F32, tag="mask")
        for r in range(4):
            m = masks[:, r, :]
            nc.gpsimd.memset(m, 1.0)
            # keep where q_local - k_local >= 0: y - x - 128*r >= 0
            nc.gpsimd.affine_select(
                out=m, in_=m, compare_op=mybir.AluOpType.is_ge, fill=0.0,
                base=-128 * r, channel_multiplier=-1, pattern=[[1, 512]],
            )
        onescol = mask_pool.tile([128, NT], F32, tag="onescol")
        nc.vector.memset(onescol[:], 1.0)
        for h in range(HPC):
            nc.vector.tensor_copy(v_sb[h][:, :, 64], onescol[:])

        # ---- Phase 1: qkv_T = wqkvT.T-stationary @ xT-moving.
        # V transposes are interleaved right after each v-tile chunk so the
        # PE never has a long regular-matmul-free window (keeps the HAM
        # clock governor at full rate into phase 2).
        with tc.tile_pool(name="x", bufs=1) as x_pool, \
             tc.tile_pool(name="wq", bufs=1) as w_pool, \
             tc.tile_pool(name="mmps", bufs=3, space="PSUM") as mm_ps, \
             tc.tile_pool(name="trps", bufs=3, space="PSUM") as tr_ps:
            xt = [x_pool.tile([128, N], F32R, tag=f"x{ct}", name=f"xt{ct}") for ct in range(CT)]
            wq = [w_pool.tile([128, 3 * HD], F32R, tag=f"w{ct}", name=f"wq{ct}") for ct in range(CT)]
            for ct in range(CT):
                nc.sync.dma_start(wq[ct][:], wq_r[ct])
            # chunked x loads: the first matmul group only needs the first
            # 512-column slice of each c-tile, not the whole 1 MB tile.
            for nch in range(QCH):
                for ct in range(CT):
                    nc.sync.dma_start(
                        xt[ct][:, nch * 512:(nch + 1) * 512],
                        xT_r[ct][:, nch * 512:(nch + 1) * 512])
            for jt in range(6):
                for nch in range(QCH):
                    ps = mm_ps.tile([128, 512], F32, tag="mm")
                    for ct in range(CT):
                        nc.tensor.matmul(
                            ps[:],
                            wq[ct][:, jt * 128:(jt + 1) * 128],
                            xt[ct][:, nch * 512:(nch + 1) * 512],
                            start=(ct == 0), stop=(ct == CT - 1),
                        )
                    nc.scalar.activation(
                        qkvT[jt][:, nch * 512:(nch + 1) * 512], ps[:], AF.Copy)
                    if jt >= 4:
                        hp = jt - 4
                        for nt in range(4 * nch, 4 * nch + 4):
                            pst = tr_ps.tile([128, 128], F32, tag="tr")
                            nc.tensor.transpose(
                                pst[:], qkvT[jt][:, nt * 128:(nt + 1) * 128],
                                identity[:])
                            nc.vector.tensor_copy(
                                v_sb[2 * hp][:, nt, 0:64], pst[:, 0:64])
                            nc.vector.tensor_copy(
                                v_sb[2 * hp + 1][:, nt, 0:64], pst[:, 64:128])

        # ---- Phase 2: causal attention, head-pair at a time ----
        with tc.tile_pool(name="p", bufs=6) as p_pool, \
             tc.tile_pool(name="avsb", bufs=3) as r_pool, \
             tc.tile_pool(name="bcast", bufs=3) as bc_pool, \
             tc.tile_pool(name="sps", bufs=3, space="PSUM") as s_ps, \
             tc.tile_pool(name="avps", bufs=2, space="PSUM") as av_ps, \
             tc.tile_pool(name="bcps", bufs=1, space="PSUM") as bc_ps:
            attention_phase(nc, tc, qkvT, v_sb, masks, yT, ones64,
                            s_ps, av_ps, bc_ps, p_pool, r_pool, bc_pool)

        # ---- Phase 3: partial out-projection ----
        with tc.tile_pool(name="prps", bufs=4, space="PSUM") as pr_ps, \
             tc.tile_pool(name="wp", bufs=1) as wp_pool, \
             tc.tile_pool(name="o", bufs=4) as o_pool:
            wp = [wp_pool.tile([128, C], F32R, tag=f"wp{ht}", name=f"wp{ht}")
                  for ht in range(2)]
            for ht in range(2):
                nc.sync.dma_start(wp[ht][:], wp_r[ht])
            for nt in range(NT):
                pso = [pr_ps.tile([128, 512], F32, tag="pr", name="pso")
                       for _ in range(2)]
                for ht in range(2):
                    for cok in range(2):
                        nc.tensor.matmul(
                            pso[cok][:],
                            yT[:, ht, nt * 128:(nt + 1) * 128],
                            wp[ht][:, cok * 512:(cok + 1) * 512],
                            start=(ht == 0), stop=(ht == 1),
                        )
                for cok in range(2):
                    ot = o_pool.tile([128, 512], F32, tag="ot")
                    nc.vector.tensor_copy(ot[:], pso[cok][:])
                    nc.sync.dma_start(
                        out_r[nt, :, cok * 512:(cok + 1) * 512], ot[:])

    nc.compile()
    return nc


def attention_phase(nc, tc, qkvT, v_sb, masks, yT, ones64, s_ps, av_ps,
                    bc_ps, p_pool, r_pool, bc_pool):
    def make_pts(hp, qc, kt, qs):
        """S^T matmuls + exp for one (pair, k-tile); head pair concurrent
        in PE row groups 0-63 / 64-127."""
        q_t, k_t = qkvT[hp], qkvT[2 + hp]
        pss = []
        for po in range(2):
            o = 64 * po
            ps = s_ps.tile([128, 512], F32, tag="s")
            nc.tensor.matmul(
                ps[:],
                k_t[o:o + 64, kt * 128:(kt + 1) * 128],
                q_t[o:o + 64, qs],
                start=True, stop=True,
            )
            pss.append(ps)
        pts = []
        for po in range(2):
            pt = p_pool.tile([128, 512], F32R, tag="pt")
            nc.scalar.activation(pt[:], pss[po][:], AF.Exp, scale=SCALE)
            if kt >= 4 * qc:
                nc.vector.tensor_mul(pt[:], pt[:], masks[:, kt - 4 * qc, :])
            pts.append(pt)
        return pts

    for hp in range(2):
        for qc in range(QCH):
            nkt = 4 * (qc + 1)
            qs = slice(qc * 512, (qc + 1) * 512)
            pav = [av_ps.tile([65, 512], F32, tag=f"av{po}", name=f"pav{po}")
                   for po in range(2)]
            # software pipeline: AV for k-tile kt issues after S for kt+1,
            # so PE never idles waiting on the exp (ScalarE) of the same kt.
            prev = make_pts(hp, qc, 0, qs)
            for kt in range(1, nkt + 1):
                cur = make_pts(hp, qc, kt, qs) if kt < nkt else None
                for po in range(2):
                    nc.tensor.matmul(
                        pav[po][:],
                        v_sb[2 * hp + po][:, kt - 1, :],
                        prev[po][:],
                        start=(kt == 1), stop=(kt == nkt),
                    )
                prev = cur
            for po in range(2):
                # evict accumulator to SBUF immediately (frees the PSUM
                # slot so the next chunk's AV can start), then normalize:
                # rows 0-63 x broadcast(1/row64).
                av = r_pool.tile([65, 512], F32R, tag="avsb")
                nc.vector.tensor_copy(av[:], pav[po][:])
                pbc = bc_ps.tile([64, 512], F32, tag="pbc")
                nc.tensor.matmul(pbc[:], ones64[64:65, :], av[64:65, :],
                                 start=True, stop=True)
                bc = bc_pool.tile([64, 512], F32, tag="bc")
                nc.vector.reciprocal_approx_accurate(
                    bc[:], pbc[:], scratch=bc_pool.tile(
                        [64, 512], F32, tag="bcs", name="bcs")[:])
                nc.vector.tensor_mul(
                    yT[64 * po:64 * po + 64, hp, qs],
                    av[0:64, :], bc[:])


_NC = None


def _get_nc():
    global _NC
    if _NC is None:
        _NC = build_nc()
    return _NC


def make_in_maps(x, w_qkv, w_proj):
    x = np.asarray(x, dtype=np.float32)
    w_qkv = np.asarray(w_qkv, dtype=np.float32)
    w_proj = np.asarray(w_proj, dtype=np.float32)
    xT = [np.ascontiguousarray(x[b].T) for b in range(B)]
    in_maps = []
    for c in range(NCORES):
        b, g = divmod(c, NCORES // B)
        rows = np.concatenate(
            [np.arange(s * C + g * HD, s * C + (g + 1) * HD) for s in range(3)])
        wqkvT = np.ascontiguousarray(w_qkv[rows, :].T)
        wpT = np.ascontiguousarray(w_proj[:, g * HD:(g + 1) * HD].T)
        in_maps.append({"xT": xT[b], "wqkvT": wqkvT, "wpT": wpT})
    return in_maps


def assemble(results, b_proj):
    b_proj = np.asarray(b_proj, dtype=np.float32)
    out = np.zeros((B, N, C), dtype=np.float32)
    for c in range(NCORES):
        b = c // (NCORES // B)
        out[b] += results[c]["out"]
    out += b_proj[None, None, :]
    return out


def kernel(x, w_qkv, w_proj, b_proj):
    nc = _get_nc()
    in_maps = make_in_maps(x, w_qkv, w_proj)
    res = run_bass_kernel_spmd(nc, in_maps, core_ids=list(range(NCORES)))
    return assemble(res.results, b_proj)


# revision 23
# speedup vs baseline: 1.4669x; 1.0922x over previous
"""Bass/Tile Trainium2 kernel for dense causal multi-head attention.

Problem: x[2,2048,1024] -> qkv (w_qkv [3072,1024]) -> 16-head causal
attention -> out proj (w_proj [1024,1024], b_proj) -> [2,2048,1024].

Sharding over 8 NeuronCores: data-parallel over batch (2) x
tensor-parallel over heads (4 groups of 4 heads). Each core computes its
768-row slice of the qkv projection, causal attention for its 4 heads,
and a partial output projection over its 256 head-dim columns. The
all-reduce after proj is realized host-side at gather time (sum of 4
partials per batch) together with the bias add.

On-core layout: activations kept transposed ([feature, seq]) so that
  * scores are computed directly as S^T = K_tile^T-stationary @ Q-moving
    (no P transposes anywhere),
  * softmax reduction over keys happens via a ones-column appended to V
    (denominator falls out of the same PE accumulation as attn@V),
  * head pairs sit at partition offsets 0/64 and their K=64 score
    matmuls run concurrently in different PE row groups.

Matmuls run in float32r (fp32 storage, ~1e-4 matmul precision, full
PE rate at N>=256); everything else is plain fp32.

Phase 2 is ScalarE-bound (exp of the score tiles), so all PE work that
does not depend on the running softmax is interleaved into the exp
shadow: pair 1's qkv projection + V transposes run inside pair 0's
attention loop, and the output projection runs inside pair 1's. That
also keeps the PE busy enough that the HAM clock governor stays at full
rate. wqkvT is laid out pair-major ([q01|k01|v01|q23|k23|v23]) so each
pair's weights are one contiguous 384-column slice.
"""

import sys
from contextlib import ExitStack

if "/opt/trn_rl_repo" not in sys.path:
    sys.path.insert(0, "/opt/trn_rl_repo")

import numpy as np

import concourse.bass as bass
import concourse.tile as tile
from concourse import bacc, mybir
from concourse.bass_utils import run_bass_kernel_spmd
from concourse.masks import make_identity

F32 = mybir.dt.float32
F32R = mybir.dt.float32r
AF = mybir.ActivationFunctionType

B, N, C = 2, 2048, 1024
H_TOT, D = 16, 64
NCORES = 8
HPC = H_TOT // (NCORES // B)  # heads per core = 4
HD = HPC * D                  # 256 per-core head-dim columns
CT = C // 128                 # 8 contraction tiles
NT = N // 128                 # 16 seq tiles
QCH = N // 512                # 4 query chunks of 512
SCALE = float(D) ** -0.5


class Ctx:
    """Shared build state."""
    pass


def _gemm_units(g, w_tiles, col0, dest, dest_slice_of, mm_pool, mm_tag,
                mm_bufs, tr_pool, tr_tag, tr_bufs, evict_engine):
    """Filler units for one [128-col j-tile] x N GEMM: per 512-query chunk,
    4 units of 2 accumulating matmuls + 1 evict unit (+ V transposes)."""
    nc = g.nc
    units = []
    for nch in range(QCH):
        cell = {}
        ns = slice(nch * 512, (nch + 1) * 512)

        def mk_mm(cts, nch=nch, ns=ns, cell=cell):
            def u():
                if "ps" not in cell:
                    cell["ps"] = mm_pool.tile([128, 512], F32, tag=mm_tag,
                                              bufs=mm_bufs, name="gps")
                for ct in cts:
                    nc.tensor.matmul(
                        cell["ps"][:],
                        w_tiles[ct][:, col0:col0 + 128],
                        g.xt[ct][:, ns],
                        start=(ct == 0), stop=(ct == CT - 1),
                    )
            return u

        def mk_evict(nch=nch, ns=ns, cell=cell):
            def u():
                if evict_engine == "act":
                    nc.scalar.activation(dest[:, ns], cell["ps"][:], AF.Copy)
                else:
                    nc.vector.tensor_copy(dest[:, ns], cell["ps"][:])
            return u

        units.append(mk_mm([0, 1]))
        units.append(mk_mm([2, 3]))
        units.append(mk_mm([4, 5]))
        units.append(mk_mm([6, 7]))
        units.append(mk_evict())
        if dest_slice_of is not None:
            hp = dest_slice_of
            for nt in range(4 * nch, 4 * nch + 4):
                def tr(nt=nt, hp=hp):
                    pst = tr_pool.tile([128, 128], F32, tag=tr_tag,
                                       bufs=tr_bufs, name="pst")
                    nc.tensor.transpose(
                        pst[:], dest[:, nt * 128:(nt + 1) * 128], g.identity[:])
                    nc.vector.tensor_copy(
                        g.v_sb[2 * hp][:, nt, 0:64], pst[:, 0:64])
                    nc.vector.tensor_copy(
                        g.v_sb[2 * hp + 1][:, nt, 0:64], pst[:, 64:128])
                units.append(tr)
    return units


def _proj_units(g, qc, psum_pool, o_pool):
    """Filler units for the output projection of seq tiles in chunk qc."""
    nc = g.nc
    units = []
    for nt in range(4 * qc, 4 * qc + 4):
        cell = {}

        def mk_mm(ht, nt=nt, cell=cell):
            def u():
                if "ps" not in cell:
                    cell["ps"] = [psum_pool.tile([128, 512], F32, tag="pr",
                                                 bufs=2, name="pso")
                                  for _ in range(2)]
                for cok in range(2):
                    nc.tensor.matmul(
                        cell["ps"][cok][:],
                        g.yT[:, ht, nt * 128:(nt + 1) * 128],
                        g.wp[ht][:, cok * 512:(cok + 1) * 512],
                        start=(ht == 0), stop=(ht == 1),
                    )
            return u

        def mk_out(cok, nt=nt, cell=cell):
            def u():
                ot = o_pool.tile([128, 512], F32, tag="ot", name="ot")
                nc.vector.tensor_copy(ot[:], cell["ps"][cok][:])
                nc.sync.dma_start(
                    g.out_r[nt, :, cok * 512:(cok + 1) * 512], ot[:])
            return u

        units.extend([mk_mm(0), mk_mm(1), mk_out(0), mk_out(1)])
    return units


def _attention_pair(g, hp, q_t, k_t, fillers, s_ps, av_ps, bc_ps,
                    p_pool, r_pool, bc_pool, chunk_cb=None):
    """Causal attention for head pair hp, popping filler units into the
    exp (ScalarE) shadow of each k-tile step."""
    nc = g.nc
    state = {"fi": 0}

    def pop(n=2):
        for _ in range(n):
            if state["fi"] < len(fillers):
                fillers[state["fi"]]()
                state["fi"] += 1

    def make_pts(qc, kt, qs):
        pss = []
        for po in range(2):
            o = 64 * po
            ps = s_ps.tile([128, 512], F32, tag="s", name="pss")
            nc.tensor.matmul(
                ps[:],
                k_t[o:o + 64, kt * 128:(kt + 1) * 128],
                q_t[o:o + 64, qs],
                start=True, stop=True,
            )
            pss.append(ps)
        pts = []
        for po in range(2):
            pt = p_pool.tile([128, 512], F32R, tag="pt", name="pt")
            nc.scalar.activation(pt[:], pss[po][:], AF.Exp, scale=SCALE)
            if kt >= 4 * qc:
                nc.vector.tensor_mul(pt[:], pt[:], g.masks[:, kt - 4 * qc, :])
            pts.append(pt)
        return pts

    for qc in range(QCH):
        nkt = 4 * (qc + 1)
        qs = slice(qc * 512, (qc + 1) * 512)
        pav = [av_ps.tile([65, 512], F32, tag=f"av{po}", bufs=1,
                          name=f"pav{po}") for po in range(2)]
        # software pipeline: AV for k-tile kt-1 issues after S for kt, so
        # the PE never idles waiting on the exp of the same k-tile.
        prev = make_pts(qc, 0, qs)
        for kt in range(1, nkt + 1):
            cur = make_pts(qc, kt, qs) if kt < nkt else None
            for po in range(2):
                nc.tensor.matmul(
                    pav[po][:],
                    g.v_sb[2 * hp + po][:, kt - 1, :],
                    prev[po][:],
                    start=(kt == 1), stop=(kt == nkt),
                )
            pop(2)
            prev = cur
        for po in range(2):
            # evict the accumulator to SBUF (frees the PSUM slot), then
            # normalize rows 0-63 by broadcast(1/row64) via a rank-1 PE
            # broadcast + fast reciprocal.
            av = r_pool.tile([65, 512], F32R, tag="avsb", name="avsb")
            nc.vector.tensor_copy(av[:], pav[po][:])
            pbc = bc_ps.tile([64, 512], F32, tag="pbc", name="pbc")
            nc.tensor.matmul(pbc[:], g.ones64[64:65, :], av[64:65, :],
                             start=True, stop=True)
            bc = bc_pool.tile([64, 512], F32, tag="bc", name="bc")
            nc.vector.reciprocal_approx_accurate(
                bc[:], pbc[:],
                scratch=bc_pool.tile([64, 512], F32, tag="bcs", name="bcs")[:])
            nc.vector.tensor_mul(
                g.yT[64 * po:64 * po + 64, hp, qs], av[0:64, :], bc[:])
            pop(1)
        if chunk_cb is not None:
            chunk_cb(qc)
    while state["fi"] < len(fillers):
        fillers[state["fi"]]()
        state["fi"] += 1


def build_nc():
    nc = bacc.Bacc("TRN2", target_bir_lowering=False, debug=False)
    xT = nc.dram_tensor("xT", [C, N], F32R, kind="ExternalInput").ap()
    wqkvT = nc.dram_tensor("wqkvT", [C, 3 * HD], F32R, kind="ExternalInput").ap()
    wpT = nc.dram_tensor("wpT", [HD, C], F32R, kind="ExternalInput").ap()
    out = nc.dram_tensor("out", [N, C], F32, kind="ExternalOutput").ap()

    xT_r = xT.rearrange("(ct p) n -> ct p n", p=128)
    wq_r = wqkvT.rearrange("(ct p) j -> ct p j", p=128)
    wp_r = wpT.rearrange("(ht p) co -> ht p co", p=128)

    g = Ctx()
    g.nc = nc
    g.out_r = out.rearrange("(nt p) co -> nt p co", p=128)

    with tile.TileContext(nc) as tc, ExitStack() as ctx:
        const = ctx.enter_context(tc.tile_pool(name="const", bufs=1))
        qkv_pool = ctx.enter_context(tc.tile_pool(name="qkv", bufs=1))
        yT_pool = ctx.enter_context(tc.tile_pool(name="yT", bufs=1))
        v_pool = ctx.enter_context(tc.tile_pool(name="v", bufs=1))
        mask_pool = ctx.enter_context(tc.tile_pool(name="mask", bufs=1))

        g.identity = const.tile([128, 128], F32, tag="id")
        make_identity(nc, g.identity[:])
        ones64f = const.tile([128, 64], F32, tag="ones64f")
        nc.vector.memset(ones64f[:], 1.0)
        g.ones64 = const.tile([128, 64], F32R, tag="ones64")
        nc.vector.tensor_copy(g.ones64[:], ones64f[:])

        # q/k tiles per pair, [d-of-pair(128), N]
        q_t = [qkv_pool.tile([128, N], F32R, tag=f"q{hp}", name=f"qT{hp}")
               for hp in range(2)]
        k_t = [qkv_pool.tile([128, N], F32R, tag=f"k{hp}", name=f"kT{hp}")
               for hp in range(2)]
        g.yT = yT_pool.tile([128, 2, N], F32R, tag="yT")
        # V per head: [k-partition, kt, 65]; col 64 = ones (denominator).
        g.v_sb = [v_pool.tile([128, NT, 65], F32R, tag=f"v{h}", name=f"v{h}")
                  for h in range(HPC)]
        # Causal masks for the 4 diagonal positions of a 512-query chunk.
        g.masks = mask_pool.tile([128, 4, 512], F32, tag="mask")
        for r in range(4):
            m = g.masks[:, r, :]
            nc.gpsimd.memset(m, 1.0)
            # keep where q_local - k_local >= 0: y - x - 128*r >= 0
            nc.gpsimd.affine_select(
                out=m, in_=m, compare_op=mybir.AluOpType.is_ge, fill=0.0,
                base=-128 * r, channel_multiplier=-1, pattern=[[1, 512]],
            )
        onescol = mask_pool.tile([128, NT], F32, tag="onescol")
        nc.vector.memset(onescol[:], 1.0)
        for h in range(HPC):
            nc.vector.tensor_copy(g.v_sb[h][:, :, 64], onescol[:])

        with tc.tile_pool(name="x", bufs=1) as x_pool, \
             tc.tile_pool(name="wb", bufs=1) as wb_pool:
            g.xt = [x_pool.tile([128, N], F32R, tag=f"x{ct}", name=f"xt{ct}")
                    for ct in range(CT)]
            wb = [wb_pool.tile([128, 384], F32R, tag=f"wb{ct}", name=f"wb{ct}")
                  for ct in range(CT)]

            # ---- Phase 1a: pair-0 qkv + V0 transposes ----
            with tc.tile_pool(name="wa", bufs=1) as wa_pool, \
                 tc.tile_pool(name="vt0", bufs=1) as vt0_pool, \
                 tc.tile_pool(name="mmps", bufs=3, space="PSUM") as mm_ps:
                wa = [wa_pool.tile([128, 384], F32R, tag=f"wa{ct}",
                                   name=f"wa{ct}") for ct in range(CT)]
                for ct in range(CT):
                    nc.sync.dma_start(wa[ct][:], wq_r[ct][:, 0:384])
                for ct in range(CT):
                    nc.sync.dma_start(wb[ct][:], wq_r[ct][:, 384:768])
                for nch in range(QCH):
                    for ct in range(CT):
                        nc.sync.dma_start(
                            g.xt[ct][:, nch * 512:(nch + 1) * 512],
                            xT_r[ct][:, nch * 512:(nch + 1) * 512])
                vt0 = vt0_pool.tile([128, N], F32, tag="vt0")
                for u in _gemm_units(g, wa, 0, q_t[0], None,
                                     mm_ps, "mm", 3, mm_ps, "tr1a", 2, "act"):
                    u()
                for u in _gemm_units(g, wa, 128, k_t[0], None,
                                     mm_ps, "mm", 3, mm_ps, "tr1a", 2, "act"):
                    u()
                for u in _gemm_units(g, wa, 256, vt0, 0,
                                     mm_ps, "mm", 3, mm_ps, "tr1a", 2, "act"):
                    u()

            # ---- Phase 2a: pair-0 attention; pair-1 qkv in exp shadow ----
            with tc.tile_pool(name="vt1", bufs=1) as vt1_pool, \
                 tc.tile_pool(name="p", bufs=6) as p_pool, \
                 tc.tile_pool(name="avsb", bufs=3) as r_pool, \
                 tc.tile_pool(name="bcast", bufs=2) as bc_pool, \
                 tc.tile_pool(name="sps", bufs=3, space="PSUM") as s_ps, \
                 tc.tile_pool(name="avps", bufs=1, space="PSUM") as av_ps, \
                 tc.tile_pool(name="bcps", bufs=1, space="PSUM") as bc_ps:
                vt1 = vt1_pool.tile([128, N], F32, tag="vt1")
                fillers = []
                fillers += _gemm_units(g, wb, 0, q_t[1], None,
                                       bc_ps, "mm", 1, bc_ps, "tr", 1, "dve")
                fillers += _gemm_units(g, wb, 128, k_t[1], None,
                                       bc_ps, "mm", 1, bc_ps, "tr", 1, "dve")
                fillers += _gemm_units(g, wb, 256, vt1, 1,
                                       bc_ps, "mm", 1, bc_ps, "tr", 1, "dve")
                _attention_pair(g, 0, q_t[0], k_t[0], fillers,
                                s_ps, av_ps, bc_ps, p_pool, r_pool, bc_pool)

        # ---- Phase 2b: pair-1 attention; out-projection in exp shadow ----
        with tc.tile_pool(name="wp", bufs=1) as wp_pool, \
             tc.tile_pool(name="o", bufs=4) as o_pool, \
             tc.tile_pool(name="p2", bufs=6) as p_pool, \
             tc.tile_pool(name="avsb2", bufs=3) as r_pool, \
             tc.tile_pool(name="bcast2", bufs=2) as bc_pool, \
             tc.tile_pool(name="sps2", bufs=3, space="PSUM") as s_ps, \
             tc.tile_pool(name="avps2", bufs=1, space="PSUM") as av_ps, \
             tc.tile_pool(name="bcps2", bufs=1, space="PSUM") as bc_ps, \
             tc.tile_pool(name="prps", bufs=1, space="PSUM") as pr_ps:
            g.wp = [wp_pool.tile([128, C], F32R, tag=f"wp{ht}", name=f"wp{ht}")
                    for ht in range(2)]
            for ht in range(2):
                nc.sync.dma_start(g.wp[ht][:], wp_r[ht])
            fillers = []

            def chunk_cb(qc):
                fillers.extend(_proj_units(g, qc, pr_ps, o_pool))

            _attention_pair(g, 1, q_t[1], k_t[1], fillers,
                            s_ps, av_ps, bc_ps, p_pool, r_pool, bc_pool,
                            chunk_cb=chunk_cb)

    nc.compile()
    return nc


_NC = None


def _get_nc():
    global _NC
    if _NC is None:
        _NC = build_nc()
    return _NC


def make_in_maps(x, w_qkv, w_proj):
    x = np.asarray(x, dtype=np.float32)
    w_qkv = np.asarray(w_qkv, dtype=np.float32)
    w_proj = np.asarray(w_proj, dtype=np.float32)
    xT = [np.ascontiguousarray(x[b].T) for b in range(B)]
    in_maps = []
    for c in range(NCORES):
        b, g = divmod(c, NCORES // B)
        # pair-major row order: [q01 | k01 | v01 | q23 | k23 | v23]
        rows = []
        for hp in range(2):
            for s in range(3):  # q, k, v blocks of w_qkv
                base = s * C + g * HD + hp * 2 * D
                rows.append(np.arange(base, base + 2 * D))
        rows = np.concatenate(rows)
        wqkvT = np.ascontiguousarray(w_qkv[rows, :].T)
        wpT = np.ascontiguousarray(w_proj[:, g * HD:(g + 1) * HD].T)
        in_maps.append({"xT": xT[b], "wqkvT": wqkvT, "wpT": wpT})
    return in_maps


def assemble(results, b_proj):
    b_proj = np.asarray(b_proj, dtype=np.float32)
    out = np.zeros((B, N, C), dtype=np.float32)
    for c in range(NCORES):
        b = c // (NCORES // B)
        out[b] += results[c]["out"]
    out += b_proj[None, None, :]
    return out


def kernel(x, w_qkv, w_proj, b_proj):
    nc = _get_nc()
    in_maps = make_in_maps(x, w_qkv, w_proj)
    res = run_bass_kernel_spmd(nc, in_maps, core_ids=list(range(NCORES)))
    return assemble(res.results, b_proj)


# revision 26
# speedup vs baseline: 1.5910x; 1.0846x over previous
"""Bass/Tile Trainium2 kernel for dense causal multi-head attention.

Problem: x[2,2048,1024] -> qkv (w_qkv [3072,1024]) -> 16-head causal
attention -> out proj (w_proj [1024,1024], b_proj) -> [2,2048,1024].

Sharding over 8 NeuronCores: data-parallel over batch (2) x
tensor-parallel over heads (4 groups of 4 heads). Each core computes its
768-row slice of the qkv projection, causal attention for its 4 heads,
and a partial output projection over its 256 head-dim columns. The
all-reduce after proj is realized host-side at gather time (sum of 4
partials per batch) together with the bias add.

On-core layout: activations kept transposed ([feature, seq]) so that
  * scores are computed directly as S^T = K_tile^T-stationary @ Q-moving
    (no P transposes anywhere),
  * softmax reduction over keys happens via a ones-column appended to V
    (denominator falls out of the same PE accumulation as attn@V),
  * head pairs sit at partition offsets 0/64 and their K=64 score
    matmuls run concurrently in different PE row groups.

Matmuls run in float32r (fp32 storage, ~1e-4 matmul precision, full
PE rate at N>=256); everything else is plain fp32.

Phase 2 is ScalarE-bound (exp of the score tiles), so all PE work that
does not depend on the running softmax is interleaved into the exp
shadow: pair 1's qkv projection + V transposes run inside pair 0's
attention loop, and the output projection runs inside pair 1's. That
also keeps the PE busy enough that the HAM clock governor stays at full
rate. wqkvT is laid out pair-major ([q01|k01|v01|q23|k23|v23]) so each
pair's weights are one contiguous 384-column slice.
"""

import sys
from contextlib import ExitStack

if "/opt/trn_rl_repo" not in sys.path:
    sys.path.insert(0, "/opt/trn_rl_repo")

import numpy as np

import concourse.bass as bass
import concourse.tile as tile
from concourse import bacc, mybir
from concourse.bass_utils import run_bass_kernel_spmd
from concourse.masks import make_identity

F32 = mybir.dt.float32
F32R = mybir.dt.float32r
AF = mybir.ActivationFunctionType

B, N, C = 2, 2048, 1024
H_TOT, D = 16, 64
NCORES = 8
HPC = H_TOT // (NCORES // B)  # heads per core = 4
HD = HPC * D                  # 256 per-core head-dim columns
CT = C // 128                 # 8 contraction tiles
NT = N // 128                 # 16 seq tiles
QCH = N // 512                # 4 query chunks of 512
SCALE = float(D) ** -0.5


class Ctx:
    """Shared build state."""
    pass


def _gemm_units(g, w_tiles, col0, dest, dest_slice_of, mm_pool, mm_tag,
                mm_bufs, tr_pool, tr_tag, tr_bufs, evict_engine):
    """Filler units for one [128-col j-tile] x N GEMM: per 512-query chunk,
    4 units of 2 accumulating matmuls + 1 evict unit (+ V transposes)."""
    nc = g.nc
    units = []
    for nch in range(QCH):
        cell = {}
        ns = slice(nch * 512, (nch + 1) * 512)

        def mk_mm(cts, nch=nch, ns=ns, cell=cell):
            def u():
                if "ps" not in cell:
                    cell["ps"] = mm_pool.tile([128, 512], F32, tag=mm_tag,
                                              bufs=mm_bufs, name="gps")
                for ct in cts:
                    nc.tensor.matmul(
                        cell["ps"][:],
                        w_tiles[ct][:, col0:col0 + 128],
                        g.xt[ct][:, ns],
                        start=(ct == 0), stop=(ct == CT - 1),
                    )
            return u

        def mk_evict(nch=nch, ns=ns, cell=cell):
            def u():
                if evict_engine == "act":
                    nc.scalar.activation(dest[:, ns], cell["ps"][:], AF.Copy)
                else:
                    nc.vector.tensor_copy(dest[:, ns], cell["ps"][:])
            return u

        units.append(mk_mm([0, 1]))
        units.append(mk_mm([2, 3]))
        units.append(mk_mm([4, 5]))
        units.append(mk_mm([6, 7]))
        units.append(mk_evict())
        if dest_slice_of is not None:
            hp = dest_slice_of
            for nt in range(4 * nch, 4 * nch + 4):
                def tr(nt=nt, hp=hp):
                    pst = tr_pool.tile([128, 128], F32, tag=tr_tag,
                                       bufs=tr_bufs, name="pst")
                    nc.tensor.transpose(
                        pst[:], dest[:, nt * 128:(nt + 1) * 128], g.identity[:])
                    # [v_even | v_odd] -> cols {0:64, 65:129} of the pair tile
                    vd = g.v_sb[hp][:, nt, :]
                    nc.vector.tensor_copy(
                        vd.rearrange("p (b c) -> p b c", b=2)[:, :, 0:64],
                        pst[:].rearrange("p (b c) -> p b c", b=2))
                units.append(tr)
    return units


def _proj_units(g, qc, psum_pool, o_pool):
    """Filler units for the output projection of seq tiles in chunk qc."""
    nc = g.nc
    units = []
    for nt in range(4 * qc, 4 * qc + 4):
        cell = {}

        def mk_mm(ht, nt=nt, cell=cell):
            def u():
                if "ps" not in cell:
                    cell["ps"] = [psum_pool.tile([128, 512], F32, tag="pr",
                                                 bufs=2, name="pso")
                                  for _ in range(2)]
                for cok in range(2):
                    nc.tensor.matmul(
                        cell["ps"][cok][:],
                        g.yT[:, ht, nt * 128:(nt + 1) * 128],
                        g.wp[ht][:, cok * 512:(cok + 1) * 512],
                        start=(ht == 0), stop=(ht == 1),
                    )
            return u

        def mk_out(cok, nt=nt, cell=cell):
            def u():
                ot = o_pool.tile([128, 512], F32, tag="ot", name="ot")
                nc.vector.tensor_copy(ot[:], cell["ps"][cok][:])
                nc.sync.dma_start(
                    g.out_r[nt, :, cok * 512:(cok + 1) * 512], ot[:])
            return u

        units.extend([mk_mm(0), mk_mm(1), mk_out(0), mk_out(1)])
    return units


def _attention_pair(g, hp, q_t, k_t, fillers, s_ps, av_ps, bc_ps,
                    p_pool, r_pool, bc_pool, chunk_cb=None,
                    pre_chunk_cb=None):
    """Causal attention for head pair hp, popping filler units into the
    exp (ScalarE) shadow of each k-tile step."""
    nc = g.nc
    total_steps = sum(4 * (qc + 1) + 2 for qc in range(QCH))
    state = {"fi": 0, "step": 0}

    def pop(nsteps):
        state["step"] += nsteps
        left = total_steps - state["step"]
        avail = len(fillers) - state["fi"]
        want = avail if left <= 0 else -(-avail // (left + 1)) * nsteps
        for _ in range(min(want, avail)):
            fillers[state["fi"]]()
            state["fi"] += 1

    def make_pts(qc, kt, qs):
        pss = []
        for po in range(2):
            o = 64 * po
            ps = s_ps.tile([128, 512], F32, tag="s", name="pss")
            nc.tensor.matmul(
                ps[:],
                k_t[o:o + 64, kt * 128:(kt + 1) * 128],
                q_t[o:o + 64, qs],
                start=True, stop=True,
            )
            pss.append(ps)
        pts = []
        for po in range(2):
            pt = p_pool.tile([128, 512], F32R, tag="pt", name="pt")
            nc.scalar.activation(pt[:], pss[po][:], AF.Exp, scale=SCALE)
            if kt >= 4 * qc:
                nc.vector.tensor_mul(pt[:], pt[:], g.masks[:, kt - 4 * qc, :])
            pts.append(pt)
        return pts

    for qc in range(QCH):
        if pre_chunk_cb is not None:
            pre_chunk_cb(qc)
        nkt = 4 * (qc + 1)
        qs = slice(qc * 512, (qc + 1) * 512)
        pav = [av_ps.tile([65, 512], F32, tag=f"av{po}", bufs=1,
                          name=f"pav{po}") for po in range(2)]
        # software pipeline: AV for k-tile kt-1 issues after S for kt, so
        # the PE never idles waiting on the exp of the same k-tile.
        prev = make_pts(qc, 0, qs)
        for kt in range(1, nkt + 1):
            cur = make_pts(qc, kt, qs) if kt < nkt else None
            for po in range(2):
                nc.tensor.matmul(
                    pav[po][:],
                    g.v_sb[hp][:, kt - 1, 65 * po:65 * po + 65],
                    prev[po][:],
                    start=(kt == 1), stop=(kt == nkt),
                )
            pop(1)
            prev = cur
        for po in range(2):
            # evict the accumulator to SBUF (frees the PSUM slot), then
            # normalize rows 0-63 by broadcast(1/row64) via a rank-1 PE
            # broadcast + fast reciprocal.
            av = r_pool.tile([65, 512], F32R, tag="avsb", name="avsb")
            nc.scalar.activation(av[:], pav[po][:], AF.Copy)
            pbc = bc_ps.tile([64, 512], F32, tag="pbc", name="pbc")
            nc.tensor.matmul(pbc[:], g.ones64[64:65, :], av[64:65, :],
                             start=True, stop=True)
            bc = bc_pool.tile([64, 512], F32, tag="bc", name="bc")
            nc.vector.reciprocal_approx_fast(bc[:], pbc[:])
            nc.vector.tensor_mul(
                g.yT[64 * po:64 * po + 64, hp, qs], av[0:64, :], bc[:])
            pop(1)
        if chunk_cb is not None:
            chunk_cb(qc)
    while state["fi"] < len(fillers):
        fillers[state["fi"]]()
        state["fi"] += 1


def build_nc():
    nc = bacc.Bacc("TRN2", target_bir_lowering=False, debug=False)
    xT = nc.dram_tensor("xT", [C, N], F32R, kind="ExternalInput").ap()
    wqkvT = nc.dram_tensor("wqkvT", [C, 3 * HD], F32R, kind="ExternalInput").ap()
    wpT = nc.dram_tensor("wpT", [HD, C], F32R, kind="ExternalInput").ap()
    out = nc.dram_tensor("out", [N, C], F32, kind="ExternalOutput").ap()

    xT_r = xT.rearrange("(ct p) n -> ct p n", p=128)
    wq_r = wqkvT.rearrange("(ct p) j -> ct p j", p=128)
    wp_r = wpT.rearrange("(ht p) co -> ht p co", p=128)

    g = Ctx()
    g.nc = nc
    g.out_r = out.rearrange("(nt p) co -> nt p co", p=128)

    with tile.TileContext(nc) as tc, ExitStack() as ctx:
        const = ctx.enter_context(tc.tile_pool(name="const", bufs=1))
        qkv_pool = ctx.enter_context(tc.tile_pool(name="qkv", bufs=1))
        yT_pool = ctx.enter_context(tc.tile_pool(name="yT", bufs=1))
        v_pool = ctx.enter_context(tc.tile_pool(name="v", bufs=1))
        mask_pool = ctx.enter_context(tc.tile_pool(name="mask", bufs=1))

        g.identity = const.tile([128, 128], F32, tag="id")
        make_identity(nc, g.identity[:])
        ones64f = const.tile([128, 64], F32, tag="ones64f")
        nc.vector.memset(ones64f[:], 1.0)
        g.ones64 = const.tile([128, 64], F32R, tag="ones64")
        nc.vector.tensor_copy(g.ones64[:], ones64f[:])

        # q/k tiles per pair, [d-of-pair(128), N]
        q_t = [qkv_pool.tile([128, N], F32R, tag=f"q{hp}", name=f"qT{hp}")
               for hp in range(2)]
        k_t = [qkv_pool.tile([128, N], F32R, tag=f"k{hp}", name=f"kT{hp}")
               for hp in range(2)]
        g.yT = yT_pool.tile([128, 2, N], F32R, tag="yT")
        # V per pair: [k-partition, kt, 130] = [v_even |1| v_odd |1];
        # col 64/129 = ones (softmax denominator row of the AV matmul).
        g.v_sb = [v_pool.tile([128, NT, 130], F32R, tag=f"v{hp}",
                              name=f"v{hp}") for hp in range(2)]
        # Causal masks for the 4 diagonal positions of a 512-query chunk.
        g.masks = mask_pool.tile([128, 4, 512], F32, tag="mask")
        for r in range(4):
            m = g.masks[:, r, :]
            nc.gpsimd.memset(m, 1.0)
            # keep where q_local - k_local >= 0: y - x - 128*r >= 0
            nc.gpsimd.affine_select(
                out=m, in_=m, compare_op=mybir.AluOpType.is_ge, fill=0.0,
                base=-128 * r, channel_multiplier=-1, pattern=[[1, 512]],
            )
        onescol = mask_pool.tile([128, NT], F32, tag="onescol")
        nc.vector.memset(onescol[:], 1.0)
        for hp in range(2):
            nc.vector.tensor_copy(g.v_sb[hp][:, :, 64], onescol[:])
            nc.vector.tensor_copy(g.v_sb[hp][:, :, 129], onescol[:])

        with tc.tile_pool(name="vt1", bufs=1) as vt1_pool:
            vt1 = vt1_pool.tile([128, N], F32, tag="vt1")

            with tc.tile_pool(name="x", bufs=1) as x_pool, \
                 tc.tile_pool(name="wb", bufs=1) as wb_pool:
                g.xt = [x_pool.tile([128, N], F32R, tag=f"x{ct}",
                                    name=f"xt{ct}") for ct in range(CT)]
                wb = [wb_pool.tile([128, 384], F32R, tag=f"wb{ct}",
                                   name=f"wb{ct}") for ct in range(CT)]

                # ---- Phase 1a: pair-0 qkv + V0 transposes ----
                with tc.tile_pool(name="wa", bufs=1) as wa_pool, \
                     tc.tile_pool(name="vt0", bufs=1) as vt0_pool, \
                     tc.tile_pool(name="mmps", bufs=3, space="PSUM") as mm_ps:
                    wa = [wa_pool.tile([128, 384], F32R, tag=f"wa{ct}",
                                       name=f"wa{ct}") for ct in range(CT)]
                    for ct in range(CT):
                        nc.sync.dma_start(wa[ct][:], wq_r[ct][:, 0:384])
                    for ct in range(CT):
                        nc.sync.dma_start(g.xt[ct][:, 0:512],
                                          xT_r[ct][:, 0:512])
                    for ct in range(CT):
                        nc.sync.dma_start(wb[ct][:], wq_r[ct][:, 384:768])
                    for nch in range(1, QCH):
                        for ct in range(CT):
                            nc.sync.dma_start(
                                g.xt[ct][:, nch * 512:(nch + 1) * 512],
                                xT_r[ct][:, nch * 512:(nch + 1) * 512])
                    vt0 = vt0_pool.tile([128, N], F32, tag="vt0")
                    for u in _gemm_units(g, wa, 0, q_t[0], None, mm_ps, "mm",
                                         3, mm_ps, "tr1a", 2, "act"):
                        u()
                    for u in _gemm_units(g, wa, 128, k_t[0], None, mm_ps,
                                         "mm", 3, mm_ps, "tr1a", 2, "act"):
                        u()
                    for u in _gemm_units(g, wa, 256, vt0, 0, mm_ps, "mm",
                                         3, mm_ps, "tr1a", 2, "act"):
                        u()

                # ---- Phase 2a: pair-0 attn; pair-1 qkv in exp shadow ----
                with tc.tile_pool(name="p", bufs=6) as p_pool, \
                     tc.tile_pool(name="avsb", bufs=3) as r_pool, \
                     tc.tile_pool(name="bcast", bufs=2) as bc_pool, \
                     tc.tile_pool(name="sps", bufs=3, space="PSUM") as s_ps, \
                     tc.tile_pool(name="avps", bufs=1, space="PSUM") as av_ps, \
                     tc.tile_pool(name="bcps", bufs=1, space="PSUM") as bc_ps:
                    fillers = []
                    fillers += _gemm_units(g, wb, 0, q_t[1], None,
                                           bc_ps, "mm", 1, None, "", 0, "dve")
                    fillers += _gemm_units(g, wb, 128, k_t[1], None,
                                           bc_ps, "mm", 1, None, "", 0, "dve")
                    fillers += _gemm_units(g, wb, 256, vt1, None,
                                           bc_ps, "mm", 1, None, "", 0, "dve")
                    _attention_pair(g, 0, q_t[0], k_t[0], fillers,
                                    s_ps, av_ps, bc_ps, p_pool, r_pool,
                                    bc_pool)

            # ---- Phase 2b: pair-1 attention; V1 transposes + projection
            # in the exp shadow ----
            with tc.tile_pool(name="wp", bufs=1) as wp_pool, \
                 tc.tile_pool(name="o", bufs=4) as o_pool, \
                 tc.tile_pool(name="p2", bufs=6) as p_pool, \
                 tc.tile_pool(name="avsb2", bufs=3) as r_pool, \
                 tc.tile_pool(name="bcast2", bufs=2) as bc_pool, \
                 tc.tile_pool(name="sps2", bufs=3, space="PSUM") as s_ps, \
                 tc.tile_pool(name="avps2", bufs=1, space="PSUM") as av_ps, \
                 tc.tile_pool(name="bcps2", bufs=1, space="PSUM") as bc_ps, \
                 tc.tile_pool(name="prps", bufs=1, space="PSUM") as pr_ps:
                g.wp = [wp_pool.tile([128, C], F32R, tag=f"wp{ht}",
                                     name=f"wp{ht}") for ht in range(2)]
                for ht in range(2):
                    nc.sync.dma_start(g.wp[ht][:], wp_r[ht])

                fillers = []

                def pre_chunk_cb(qc):
                    # V1 transposes for the k-tiles this chunk will touch
                    # (must precede the AV matmuls that read v_sb[1]).
                    for nt in range(4 * qc, 4 * qc + 4):
                        pst = pr_ps.tile([128, 128], F32, tag="pr", bufs=2,
                                         name="pst")
                        nc.tensor.transpose(
                            pst[:], vt1[:, nt * 128:(nt + 1) * 128],
                            g.identity[:])
                        vd = g.v_sb[1][:, nt, :]
                        nc.vector.tensor_copy(
                            vd.rearrange("p (b c) -> p b c", b=2)[:, :, 0:64],
                            pst[:].rearrange("p (b c) -> p b c", b=2))

                def chunk_cb(qc):
                    fillers.extend(_proj_units(g, qc, pr_ps, o_pool))

                _attention_pair(g, 1, q_t[1], k_t[1], fillers,
                                s_ps, av_ps, bc_ps, p_pool, r_pool, bc_pool,
                                chunk_cb=chunk_cb, pre_chunk_cb=pre_chunk_cb)

    nc.compile()
    return nc


_NC = None


def _get_nc():
    global _NC
    if _NC is None:
        _NC = build_nc()
    return _NC


def make_in_maps(x, w_qkv, w_proj):
    x = np.asarray(x, dtype=np.float32)
    w_qkv = np.asarray(w_qkv, dtype=np.float32)
    w_proj = np.asarray(w_proj, dtype=np.float32)
    xT = [np.ascontiguousarray(x[b].T) for b in range(B)]
    in_maps = []
    for c in range(NCORES):
        b, g = divmod(c, NCORES // B)
        # pair-major row order: [q01 | k01 | v01 | q23 | k23 | v23]
        rows = []
        for hp in range(2):
            for s in range(3):  # q, k, v blocks of w_qkv
                base = s * C + g * HD + hp * 2 * D
                rows.append(np.arange(base, base + 2 * D))
        rows = np.concatenate(rows)
        wqkvT = np.ascontiguousarray(w_qkv[rows, :].T)
        wpT = np.ascontiguousarray(w_proj[:, g * HD:(g + 1) * HD].T)
        in_maps.append({"xT": xT[b], "wqkvT": wqkvT, "wpT": wpT})
    return in_maps


def assemble(results, b_proj):
    b_proj = np.asarray(b_proj, dtype=np.float32)
    out = np.zeros((B, N, C), dtype=np.float32)
    for c in range(NCORES):
        b = c // (NCORES // B)
        out[b] += results[c]["out"]
    out += b_proj[None, None, :]
    return out


def kernel(x, w_qkv, w_proj, b_proj):
    nc = _get_nc()
    in_maps = make_in_maps(x, w_qkv, w_proj)
    res = run_bass_kernel_spmd(nc, in_maps, core_ids=list(range(NCORES)))
    return assemble(res.results, b_proj)
